# revision 14
# baseline (speedup 1.0000x reference)
"""LocalAutoCorr2D Trainium2 kernel.

out[b,c,i,j,dy,dx] = sum_{y,x valid} x[b,c,4i+y,4j+x] * x[b,c,4i+y+sy,4j+x+sx]
with (sy,sx) = (dy-4, dx-4), windows 8x8 at stride 4 on a 96x96 image,
zero-padded at window boundaries.

Strategy (per core, batch-sharded over 8 cores):
  - out[s] == out[-s] (autocorr symmetry) -> only 40 canonical shift classes.
  - x is host-prepped into a PHASE-MAJOR fp16 layout [h, (r, j, c)] with
    w = 4j + r and c innermost, so every matmul rhs view (fixed r, a
    23-j window, all c) is one FLAT contiguous slice: the PE streams at
    full rate (a strided or multi-dim rhs runs at ~half rate). The 5
    vertical shifts v=0..4 are also host-stacked along the free dim, so
    DVE products never need cross-partition operands.
  - Per shift: product Q = x .* shift(x) on the Vector engine (fp16 2x
    mode, flat contiguous views; all shift offsets are multiples of C=64
    elements, so alignment is automatic). Vertical box-sum via 0/1-weight
    matmul (h on partitions), horizontal box-sum folded into PSUM
    accumulation across <=8 matmuls over flat rhs slices of Q.
  - Scalar engine evacuates PSUM -> SBUF; GpSimd queues the output DMAs.
"""

import functools
import os
import sys

import numpy as np

sys.path.insert(0, "/opt/trn_rl_repo")

import concourse.bass as bass  # noqa: E402
import concourse.bacc as bacc  # noqa: E402
import concourse.mybir as mybir  # noqa: E402
from concourse import bass_utils  # noqa: E402
from concourse.tile import TileContext  # noqa: E402

B, C, H, W = 8, 64, 96, 96
KH = KW = 8
SH = SW = 4
NH = NW = 23
NCORES = 8

JP = 24           # j' positions per r-block (w = 4j + r)
BLK = C * JP      # 1536 elements per r-block
FLAT = 4 * BLK    # 6144
NV = 5            # vertical shift copies v=0..4 stacked in the free dim
BASE = 64         # leading pad elements (AP validity for negative offsets)
TAIL = 128
XCOLS = BASE + NV * FLAT + TAIL
N_CHUNKS = [(0, 512), (512, 1024), (1024, 1472)]  # flat cols per PSUM bank
N_WARM = 30       # PE warmup matmuls issued under the input DMA

fp32 = mybir.dt.float32
fp16 = mybir.dt.float16


def _canonical_cells():
    """Map canonical shift (sy>=0, sx) -> list of output cells (dy,dx)."""
    cells = {}
    for dy in range(8):
        for dx in range(8):
            sy, sx = dy - 4, dx - 4
            key = (sy, sx) if (sy > 0 or (sy == 0 and sx >= 0)) else (-sy, -sx)
            cells.setdefault(key, []).append((dy, dx))
    assert len(cells) == 40
    return cells


def _amat_np():
    """Vertical box-sum matrices, stacked: A[h, sy*23+i] = 1 if 0<=h-4i<8-sy."""
    a = np.zeros((H, 5 * NH), np.float16)
    for sy in range(5):
        for i in range(NH):
            a[4 * i : 4 * i + 8 - sy, sy * NH + i] = 1.0
    return a


def _prep_x(xb):
    """[C,H,W] fp32 -> xa phase-major fp16 [H, XCOLS].

    xa[h, BASE + v*FLAT + (r,j,c)] = x[h+v, c, 4j+r]  (0 beyond the image)."""
    t = xb.transpose(1, 2, 0)  # [h, w, c]
    pm = t.reshape(H, JP, 4, C).transpose(0, 2, 1, 3)  # [h, r, j, c]
    flat = np.ascontiguousarray(pm.reshape(H, FLAT)).astype(np.float16)
    xa = np.zeros((H, XCOLS), np.float16)
    for v in range(NV):
        xa[0 : H - v, BASE + v * FLAT : BASE + (v + 1) * FLAT] = flat[v:H]
    return xa


def _order(cells):
    """sy=0 shifts first (their stack block lands first), then by growing
    |sx| so the PE builds backlog early; (4,0) moved to the very end so
    the PE drains on a big-Lx shift instead of starving."""
    o = sorted(cells.keys(), key=lambda s: (s[0], abs(s[1])))
    o.remove((4, 0))
    o.append((4, 0))
    return o


def build_nc():
    nc = bacc.Bacc()
    xa_dram = nc.dram_tensor("xa", [H, XCOLS], fp16, kind="ExternalInput")
    amat_dram = nc.dram_tensor("amat", [H, 5 * NH], fp16, kind="ExternalInput")
    out_dram = nc.dram_tensor("out", [8, 8, NH, NW * C], fp16,
                              kind="ExternalOutput")

    cells = _canonical_cells()
    order = _order(cells)

    with TileContext(nc) as tc:
        with (
            tc.tile_pool(name="const", bufs=1) as cpool,
            tc.tile_pool(name="q", bufs=4) as qpool,
            tc.tile_pool(name="o", bufs=3) as opool,
            tc.tile_pool(name="ps", bufs=2, space="PSUM") as ppool,
            tc.tile_pool(name="pw", bufs=1, space="PSUM") as wpool,
        ):
            amat_t = cpool.tile([H, 5 * NH], fp16)
            nc.sync.dma_start(amat_t, amat_dram[:, :])
            xa_t = cpool.tile([H, XCOLS], fp16)
            # PE warmup: keep the p-state ramped while inputs stream in
            wt = cpool.tile([H, 512], fp16)
            nc.vector.memset(wt, 0.0)
            warm_pt = wpool.tile([NH, 512], fp32)
            for _ in range(N_WARM):
                nc.tensor.matmul(warm_pt, wt[:, 0:NH], wt,
                                 start=True, stop=True)
            # chunked so the v=0 block (first consumer) lands first; v=0
            # in halves so the (0,0) square can start on the first half.
            # Issued from different engines: each engine's DGE feeds its
            # own DMA queue, so the chunks transfer in parallel instead
            # of serializing on one queue.
            bounds = [0, BASE + FLAT // 2] + \
                [BASE + v * FLAT for v in range(1, NV)] + [XCOLS]
            issuers = [nc.gpsimd, nc.sync, nc.gpsimd, nc.sync,
                       nc.gpsimd, nc.sync]
            for eng, (lo, hi) in zip(issuers, zip(bounds[:-1], bounds[1:])):
                eng.dma_start(xa_t[:, lo:hi], xa_dram[:, lo:hi])

            for (sy, sx) in order:
                s = sx % 4          # python %: s in [0,4) also for sx<0
                a = (sx - s) // 4
                hv = H - sy
                q = qpool.tile([H, FLAT], fp16, tag="q")

                def mul(flo, fhi, delta):
                    # q[h, f] = x[h, f] * x[h+sy, f+delta-sy*FLAT] on
                    # f in [flo, fhi); the sy shift is baked into the stack.
                    off = BASE + delta
                    nc.vector.tensor_mul(
                        q[0:hv, flo:fhi],
                        xa_t[0:hv, BASE + flo : BASE + fhi],
                        xa_t[0:hv, off + flo : off + fhi],
                    )

                if (sy, sx) == (0, 0):
                    # x^2 on the Act engine: frees the DVE and starts as
                    # soon as each half of the v=0 DMA chunk lands
                    for lo, hi in [(0, FLAT // 2), (FLAT // 2, FLAT)]:
                        nc.scalar.activation(
                            q[:, lo:hi], xa_t[:, BASE + lo : BASE + hi],
                            mybir.ActivationFunctionType.Square,
                        )
                else:
                    lenA = (4 - s) * BLK
                    mul(0, lenA, sy * FLAT + s * BLK + C * a)
                    if s:
                        mul(lenA, FLAT,
                            sy * FLAT + (s - 4) * BLK + C * (a + 1))

                a_k = amat_t[0:hv, sy * NH : (sy + 1) * NH]
                xlist = list(range(max(0, -sx), 8 - max(0, sx)))
                o_t = opool.tile([NH, NW * C], fp16, tag="o")
                for ci, (n0, n1) in enumerate(N_CHUNKS):
                    pt = ppool.tile([NH, n1 - n0], fp32, tag=f"ps{ci}")
                    for xi, xx in enumerate(xlist):
                        base = (xx % 4) * BLK + C * (xx // 4)
                        rhs = q[0:hv, base + n0 : base + n1]
                        nc.tensor.matmul(
                            pt, a_k, rhs,
                            start=(xi == 0), stop=(xi == len(xlist) - 1),
                        )
                    nc.scalar.copy(o_t[:, n0:n1], pt)
                for k, (dy, dx) in enumerate(cells[(sy, sx)]):
                    # alternate queues so the tail burst drains in parallel
                    eng = nc.gpsimd if k == 0 else nc.sync
                    eng.dma_start(out_dram[dy, dx], o_t)

    if not nc.is_finalized():
        nc.finalize()
    return nc


@functools.lru_cache(maxsize=1)
def _get_nc():
    return build_nc()


def _in_maps(x):
    amat = _amat_np()
    return [{"xa": _prep_x(x[b]), "amat": amat} for b in range(NCORES)]


def kernel(**inputs) -> np.ndarray:
    x = np.asarray(inputs["x"], dtype=np.float32)
    assert x.shape == (B, C, H, W)
    nc = _get_nc()
    in_maps = _in_maps(x)
    res = bass_utils.run_bass_kernel_spmd(
        nc, in_maps, core_ids=list(range(NCORES)),
        trace=bool(int(os.environ.get("KERNEL_TRACE", "0"))),
    )
    outs = np.stack([r["out"] for r in res.results])  # [B, dy, dx, i, (j c)]
    outs = outs.reshape(B, 8, 8, NH, NW, C)
    # -> [B, c, i, j, dy, dx]
    full = outs.transpose(0, 5, 3, 4, 1, 2)
    return np.ascontiguousarray(full).astype(np.float32)


if __name__ == "__main__":
    rng = np.random.default_rng(0)
    x = rng.standard_normal((B, C, H, W), dtype=np.float32)
    y = kernel(x=x)
    print("out", y.shape, y.dtype, float(np.abs(y).max()))


# revision 15
# speedup vs baseline: 1.0040x; 1.0040x over previous
"""LocalAutoCorr2D Trainium2 kernel.

out[b,c,i,j,dy,dx] = sum_{y,x valid} x[b,c,4i+y,4j+x] * x[b,c,4i+y+sy,4j+x+sx]
with (sy,sx) = (dy-4, dx-4), windows 8x8 at stride 4 on a 96x96 image,
zero-padded at window boundaries.

Strategy (per core, batch-sharded over 8 cores):
  - out[s] == out[-s] (autocorr symmetry) -> only 40 canonical shift classes.
  - x is host-prepped into a PHASE-MAJOR fp16 layout [h, (r, j, c)] with
    w = 4j + r and c innermost, so every matmul rhs view (fixed r, a
    23-j window, all c) is one FLAT contiguous slice: the PE streams at
    full rate (a strided or multi-dim rhs runs at ~half rate). The 5
    vertical shifts v=0..4 are also host-stacked along the free dim, so
    DVE products never need cross-partition operands.
  - Per shift: product Q = x .* shift(x) on the Vector engine (fp16 2x
    mode, flat contiguous views; all shift offsets are multiples of C=64
    elements, so alignment is automatic). Vertical box-sum via 0/1-weight
    matmul (h on partitions), horizontal box-sum folded into PSUM
    accumulation across <=8 matmuls over flat rhs slices of Q.
  - Scalar engine evacuates PSUM -> SBUF; GpSimd queues the output DMAs.
"""

import functools
import os
import sys

import numpy as np

sys.path.insert(0, "/opt/trn_rl_repo")

import concourse.bass as bass  # noqa: E402
import concourse.bacc as bacc  # noqa: E402
import concourse.mybir as mybir  # noqa: E402
from concourse import bass_utils  # noqa: E402
from concourse.tile import TileContext  # noqa: E402

B, C, H, W = 8, 64, 96, 96
KH = KW = 8
SH = SW = 4
NH = NW = 23
NCORES = 8

JP = 24           # j' positions per r-block (w = 4j + r)
BLK = C * JP      # 1536 elements per r-block
FLAT = 4 * BLK    # 6144
NV = 5            # vertical shift copies v=0..4 stacked in the free dim
BASE = 64         # leading pad elements (AP validity for negative offsets)
TAIL = 128
XCOLS = BASE + NV * FLAT + TAIL
N_CHUNKS = [(0, 512), (512, 1024), (1024, 1472)]  # flat cols per PSUM bank
N_WARM = 30       # PE warmup matmuls issued under the input DMA

fp32 = mybir.dt.float32
fp16 = mybir.dt.float16


def _canonical_cells():
    """Map canonical shift (sy>=0, sx) -> list of output cells (dy,dx)."""
    cells = {}
    for dy in range(8):
        for dx in range(8):
            sy, sx = dy - 4, dx - 4
            key = (sy, sx) if (sy > 0 or (sy == 0 and sx >= 0)) else (-sy, -sx)
            cells.setdefault(key, []).append((dy, dx))
    assert len(cells) == 40
    return cells


def _amat_np():
    """Vertical box-sum matrices, stacked: A[h, sy*23+i] = 1 if 0<=h-4i<8-sy."""
    a = np.zeros((H, 5 * NH), np.float16)
    for sy in range(5):
        for i in range(NH):
            a[4 * i : 4 * i + 8 - sy, sy * NH + i] = 1.0
    return a


def _prep_x(xb):
    """[C,H,W] fp32 -> xa phase-major fp16 [H, XCOLS].

    xa[h, BASE + v*FLAT + (r,j,c)] = x[h+v, c, 4j+r]  (0 beyond the image)."""
    t = xb.transpose(1, 2, 0)  # [h, w, c]
    pm = t.reshape(H, JP, 4, C).transpose(0, 2, 1, 3)  # [h, r, j, c]
    flat = np.ascontiguousarray(pm.reshape(H, FLAT)).astype(np.float16)
    xa = np.zeros((H, XCOLS), np.float16)
    for v in range(NV):
        xa[0 : H - v, BASE + v * FLAT : BASE + (v + 1) * FLAT] = flat[v:H]
    return xa


def _order(cells):
    """sy=0 shifts first (their stack block lands first), then by growing
    |sx| so the PE builds backlog early; (4,0) moved to the very end so
    the PE drains on a big-Lx shift instead of starving."""
    o = sorted(cells.keys(), key=lambda s: (s[0], abs(s[1])))
    o.remove((4, 0))
    o.append((4, 0))
    return o


def build_nc():
    nc = bacc.Bacc()
    xa_dram = nc.dram_tensor("xa", [H, XCOLS], fp16, kind="ExternalInput")
    amat_dram = nc.dram_tensor("amat", [H, 5 * NH], fp16, kind="ExternalInput")
    out_dram = nc.dram_tensor("out", [8, 8, NH, NW * C], fp16,
                              kind="ExternalOutput")

    cells = _canonical_cells()
    order = _order(cells)

    with TileContext(nc) as tc:
        with (
            tc.tile_pool(name="const", bufs=1) as cpool,
            tc.tile_pool(name="q", bufs=4) as qpool,
            tc.tile_pool(name="o", bufs=3) as opool,
            tc.tile_pool(name="ps", bufs=2, space="PSUM") as ppool,
            tc.tile_pool(name="pw", bufs=1, space="PSUM") as wpool,
        ):
            amat_t = cpool.tile([H, 5 * NH], fp16)
            nc.sync.dma_start(amat_t, amat_dram[:, :])
            xa_t = cpool.tile([H, XCOLS], fp16)
            # PE warmup: keep the p-state ramped while inputs stream in
            wt = cpool.tile([H, 512], fp16)
            nc.vector.memset(wt, 0.0)
            warm_pt = wpool.tile([NH, 512], fp32)
            for _ in range(N_WARM):
                nc.tensor.matmul(warm_pt, wt[:, 0:NH], wt,
                                 start=True, stop=True)
            # chunked so the v=0 block (first consumer) lands first; v=0
            # in halves so the (0,0) square can start on the first half.
            # Issued from different engines: each engine's DGE feeds its
            # own DMA queue, so the chunks transfer in parallel instead
            # of serializing on one queue.
            bounds = [0, BASE + FLAT // 2] + \
                [BASE + v * FLAT for v in range(1, NV)] + [XCOLS]
            issuers = [nc.gpsimd, nc.sync, nc.gpsimd, nc.sync,
                       nc.gpsimd, nc.sync]
            for eng, (lo, hi) in zip(issuers, zip(bounds[:-1], bounds[1:])):
                eng.dma_start(xa_t[:, lo:hi], xa_dram[:, lo:hi])

            for (sy, sx) in order:
                s = sx % 4          # python %: s in [0,4) also for sx<0
                a = (sx - s) // 4
                hv = H - sy
                q = qpool.tile([H, FLAT], fp16, tag="q")

                def mul(flo, fhi, delta):
                    # q[h, f] = x[h, f] * x[h+sy, f+delta-sy*FLAT] on
                    # f in [flo, fhi); the sy shift is baked into the stack.
                    off = BASE + delta
                    nc.vector.tensor_mul(
                        q[0:hv, flo:fhi],
                        xa_t[0:hv, BASE + flo : BASE + fhi],
                        xa_t[0:hv, off + flo : off + fhi],
                    )

                if (sy, sx) == (0, 0):
                    # x^2 on the Act engine: frees the DVE and starts as
                    # soon as each half of the v=0 DMA chunk lands
                    for lo, hi in [(0, FLAT // 2), (FLAT // 2, FLAT)]:
                        nc.scalar.activation(
                            q[:, lo:hi], xa_t[:, BASE + lo : BASE + hi],
                            mybir.ActivationFunctionType.Square,
                        )
                else:
                    lenA = (4 - s) * BLK
                    mul(0, lenA, sy * FLAT + s * BLK + C * a)
                    if s:
                        mul(lenA, FLAT,
                            sy * FLAT + (s - 4) * BLK + C * (a + 1))

                a_k = amat_t[0:hv, sy * NH : (sy + 1) * NH]
                xlist = list(range(max(0, -sx), 8 - max(0, sx)))
                o_t = opool.tile([NH, NW * C], fp16, tag="o")
                for ci, (n0, n1) in enumerate(N_CHUNKS):
                    pt = ppool.tile([NH, n1 - n0], fp32, tag=f"ps{ci}")
                    for xi, xx in enumerate(xlist):
                        base = (xx % 4) * BLK + C * (xx // 4)
                        rhs = q[0:hv, base + n0 : base + n1]
                        nc.tensor.matmul(
                            pt, a_k, rhs,
                            start=(xi == 0), stop=(xi == len(xlist) - 1),
                        )
                    nc.scalar.copy(o_t[:, n0:n1], pt)
                for (dy, dx) in cells[(sy, sx)]:
                    nc.gpsimd.dma_start(out_dram[dy, dx], o_t)

    if not nc.is_finalized():
        nc.finalize()
    return nc


@functools.lru_cache(maxsize=1)
def _get_nc():
    return build_nc()


def _in_maps(x):
    amat = _amat_np()
    return [{"xa": _prep_x(x[b]), "amat": amat} for b in range(NCORES)]


def kernel(**inputs) -> np.ndarray:
    x = np.asarray(inputs["x"], dtype=np.float32)
    assert x.shape == (B, C, H, W)
    nc = _get_nc()
    in_maps = _in_maps(x)
    res = bass_utils.run_bass_kernel_spmd(
        nc, in_maps, core_ids=list(range(NCORES)),
        trace=bool(int(os.environ.get("KERNEL_TRACE", "0"))),
    )
    outs = np.stack([r["out"] for r in res.results])  # [B, dy, dx, i, (j c)]
    outs = outs.reshape(B, 8, 8, NH, NW, C)
    # -> [B, c, i, j, dy, dx]
    full = outs.transpose(0, 5, 3, 4, 1, 2)
    return np.ascontiguousarray(full).astype(np.float32)


if __name__ == "__main__":
    rng = np.random.default_rng(0)
    x = rng.standard_normal((B, C, H, W), dtype=np.float32)
    y = kernel(x=x)
    print("out", y.shape, y.dtype, float(np.abs(y).max()))


# revision 16
# speedup vs baseline: 1.6968x; 1.6901x over previous
"""LocalAutoCorr2D Trainium2 kernel.

out[b,c,i,j,dy,dx] = sum_{y,x valid} x[b,c,4i+y,4j+x] * x[b,c,4i+y+sy,4j+x+sx]
with (sy,sx) = (dy-4, dx-4), windows 8x8 at stride 4 on a 96x96 image,
zero-padded at window boundaries.

Strategy (per core, batch-sharded over 8 cores):
  - out[s] == out[-s] (autocorr symmetry) -> only 40 canonical shift classes.
  - x is host-prepped into a PHASE-MAJOR fp16 layout [h, (r, j, c)] with
    w = 4j + r and c innermost, so every matmul rhs view (fixed r, a
    23-j window, all c) is one FLAT contiguous slice: the PE streams at
    full rate (a strided or multi-dim rhs runs at ~half rate). The 5
    vertical shifts v=0..4 are also host-stacked along the free dim, so
    DVE products never need cross-partition operands.
  - Per shift: product Q = x .* shift(x) on the Vector engine (fp16 2x
    mode, flat contiguous views; all shift offsets are multiples of C=64
    elements, so alignment is automatic). Vertical box-sum via 0/1-weight
    matmul (h on partitions), horizontal box-sum folded into PSUM
    accumulation across <=8 matmuls over flat rhs slices of Q.
  - Scalar engine evacuates PSUM -> SBUF; GpSimd queues the output DMAs.
"""

import functools
import os
import sys

import numpy as np

sys.path.insert(0, "/opt/trn_rl_repo")

import concourse.bass as bass  # noqa: E402
import concourse.bacc as bacc  # noqa: E402
import concourse.mybir as mybir  # noqa: E402
from concourse import bass_utils  # noqa: E402
from concourse.tile import TileContext  # noqa: E402

B, C, H, W = 8, 64, 96, 96
KH = KW = 8
SH = SW = 4
NH = NW = 23
NCORES = 8

JP = 24           # j' positions per r-block (w = 4j + r)
BLK = C * JP      # 1536 elements per r-block
FLAT = 4 * BLK    # 6144
NV = 5            # vertical shift copies v=0..4 stacked in the free dim
BASE = 64         # leading pad elements (AP validity for negative offsets)
TAIL = 128
XCOLS = BASE + NV * FLAT + TAIL
N_CHUNKS = [(0, 512), (512, 1024), (1024, 1472)]  # flat cols per PSUM bank
N_WARM = 52       # PE warmup: must bridge until the first product is ready

fp32 = mybir.dt.float32
fp16 = mybir.dt.float16


def _canonical_cells():
    """Map canonical shift (sy>=0, sx) -> list of output cells (dy,dx)."""
    cells = {}
    for dy in range(8):
        for dx in range(8):
            sy, sx = dy - 4, dx - 4
            key = (sy, sx) if (sy > 0 or (sy == 0 and sx >= 0)) else (-sy, -sx)
            cells.setdefault(key, []).append((dy, dx))
    assert len(cells) == 40
    return cells


def _amat_np():
    """Vertical box-sum matrices, stacked: A[h, sy*23+i] = 1 if 0<=h-4i<8-sy."""
    a = np.zeros((H, 5 * NH), np.float16)
    for sy in range(5):
        for i in range(NH):
            a[4 * i : 4 * i + 8 - sy, sy * NH + i] = 1.0
    return a


def _prep_x(xb):
    """[C,H,W] fp32 -> xa phase-major fp16 [H, XCOLS].

    xa[h, BASE + v*FLAT + (r,j,c)] = x[h+v, c, 4j+r]  (0 beyond the image)."""
    t = xb.transpose(1, 2, 0)  # [h, w, c]
    pm = t.reshape(H, JP, 4, C).transpose(0, 2, 1, 3)  # [h, r, j, c]
    flat = np.ascontiguousarray(pm.reshape(H, FLAT)).astype(np.float16)
    xa = np.zeros((H, XCOLS), np.float16)
    for v in range(NV):
        xa[0 : H - v, BASE + v * FLAT : BASE + (v + 1) * FLAT] = flat[v:H]
    return xa


def _order(cells):
    """sy=0 shifts first (their stack block lands first), then by growing
    |sx| so the PE builds backlog early; (4,0) moved to the very end so
    the PE drains on a big-Lx shift instead of starving."""
    o = sorted(cells.keys(), key=lambda s: (s[0], abs(s[1])))
    o.remove((4, 0))
    o.append((4, 0))
    return o


def build_nc():
    nc = bacc.Bacc()
    xa_dram = nc.dram_tensor("xa", [H, XCOLS], fp16, kind="ExternalInput")
    amat_dram = nc.dram_tensor("amat", [H, 5 * NH], fp16, kind="ExternalInput")
    out_dram = nc.dram_tensor("out", [8, 8, NH, NW * C], fp16,
                              kind="ExternalOutput")

    cells = _canonical_cells()
    order = _order(cells)

    with TileContext(nc) as tc:
        with (
            tc.tile_pool(name="const", bufs=1) as cpool,
            tc.tile_pool(name="q", bufs=4) as qpool,
            tc.tile_pool(name="o", bufs=3) as opool,
            tc.tile_pool(name="ps", bufs=2, space="PSUM") as ppool,
            tc.tile_pool(name="pw", bufs=1, space="PSUM") as wpool,
        ):
            amat_t = cpool.tile([H, 5 * NH], fp16)
            nc.sync.dma_start(amat_t, amat_dram[:, :])
            xa_t = cpool.tile([H, XCOLS], fp16)
            # PE warmup: keep the p-state ramped while inputs stream in
            wt = cpool.tile([H, 512], fp16)
            nc.vector.memset(wt, 0.0)
            warm_pt = wpool.tile([NH, 512], fp32)
            for _ in range(N_WARM):
                nc.tensor.matmul(warm_pt, wt[:, 0:NH], wt,
                                 start=True, stop=True)
            # chunked so the v=0 block (first consumer) lands first; v=0
            # in halves so the (0,0) square can start on the first half.
            # Issued from different engines: each engine's DGE feeds its
            # own DMA queue, so the chunks transfer in parallel instead
            # of serializing on one queue.
            bounds = [0, BASE + FLAT // 2] + \
                [BASE + v * FLAT for v in range(1, NV)] + [XCOLS]
            issuers = [nc.gpsimd, nc.sync, nc.gpsimd, nc.sync,
                       nc.gpsimd, nc.sync]
            for eng, (lo, hi) in zip(issuers, zip(bounds[:-1], bounds[1:])):
                eng.dma_start(xa_t[:, lo:hi], xa_dram[:, lo:hi])

            for (sy, sx) in order:
                s = sx % 4          # python %: s in [0,4) also for sx<0
                a = (sx - s) // 4
                hv = H - sy
                q = qpool.tile([H, FLAT], fp16, tag="q")

                def mul(flo, fhi, delta):
                    # q[h, f] = x[h, f] * x[h+sy, f+delta-sy*FLAT] on
                    # f in [flo, fhi); the sy shift is baked into the stack.
                    off = BASE + delta
                    nc.vector.tensor_mul(
                        q[0:hv, flo:fhi],
                        xa_t[0:hv, BASE + flo : BASE + fhi],
                        xa_t[0:hv, off + flo : off + fhi],
                    )

                if (sy, sx) == (0, 0):
                    # x^2 on the Act engine: frees the DVE and starts as
                    # soon as each half of the v=0 DMA chunk lands
                    for lo, hi in [(0, FLAT // 2), (FLAT // 2, FLAT)]:
                        nc.scalar.activation(
                            q[:, lo:hi], xa_t[:, BASE + lo : BASE + hi],
                            mybir.ActivationFunctionType.Square,
                        )
                else:
                    lenA = (4 - s) * BLK
                    mul(0, lenA, sy * FLAT + s * BLK + C * a)
                    if s:
                        mul(lenA, FLAT,
                            sy * FLAT + (s - 4) * BLK + C * (a + 1))

                a_k = amat_t[0:hv, sy * NH : (sy + 1) * NH]
                xlist = list(range(max(0, -sx), 8 - max(0, sx)))
                o_t = opool.tile([NH, NW * C], fp16, tag="o")
                for ci, (n0, n1) in enumerate(N_CHUNKS):
                    pt = ppool.tile([NH, n1 - n0], fp32, tag=f"ps{ci}")
                    for xi, xx in enumerate(xlist):
                        base = (xx % 4) * BLK + C * (xx // 4)
                        rhs = q[0:hv, base + n0 : base + n1]
                        nc.tensor.matmul(
                            pt, a_k, rhs,
                            start=(xi == 0), stop=(xi == len(xlist) - 1),
                        )
                    nc.scalar.copy(o_t[:, n0:n1], pt)
                for (dy, dx) in cells[(sy, sx)]:
                    nc.gpsimd.dma_start(out_dram[dy, dx], o_t)

    if not nc.is_finalized():
        nc.finalize()
    return nc


@functools.lru_cache(maxsize=1)
def _get_nc():
    return build_nc()


def _in_maps(x):
    amat = _amat_np()
    return [{"xa": _prep_x(x[b]), "amat": amat} for b in range(NCORES)]


def kernel(**inputs) -> np.ndarray:
    x = np.asarray(inputs["x"], dtype=np.float32)
    assert x.shape == (B, C, H, W)
    nc = _get_nc()
    in_maps = _in_maps(x)
    res = bass_utils.run_bass_kernel_spmd(
        nc, in_maps, core_ids=list(range(NCORES)),
        trace=bool(int(os.environ.get("KERNEL_TRACE", "0"))),
    )
    outs = np.stack([r["out"] for r in res.results])  # [B, dy, dx, i, (j c)]
    outs = outs.reshape(B, 8, 8, NH, NW, C)
    # -> [B, c, i, j, dy, dx]
    full = outs.transpose(0, 5, 3, 4, 1, 2)
    return np.ascontiguousarray(full).astype(np.float32)


if __name__ == "__main__":
    rng = np.random.default_rng(0)
    x = rng.standard_normal((B, C, H, W), dtype=np.float32)
    y = kernel(x=x)
    print("out", y.shape, y.dtype, float(np.abs(y).max()))
